# revision 14
# baseline (speedup 1.0000x reference)
"""Trainium2 Bass kernel for nn_ComplexValuedEngine.

Strategy (8 NeuronCores, data-parallel over cells):
- 2048 cells / 8 cores = 256 cells per core = exactly one faction per core,
  so the faction-sync mean is a purely local free-dim reduction.
- All activations live TRANSPOSED on device: [features, cells]. Every GEMM is
  then out[m_feat_tile, cells] = W_slice.T @ actT with natural-layout weights
  as the PE stationary operand and N=256 moving (full-rate bf16).
- Host folds the two complex linears (o = a - g  =>  single weight diff Dr/Di)
  and turns the shared x-row into per-feature biases u_re/u_im.
- GRU inputs are reordered so cm = [x_mag(512), h_mag(1024), tension(1)]:
  12 clean K=128 tiles plus one K=1 rank-1 matmul for the tension feature.
- Device also computes softmax partials: e = exp(tension) (via the sigmoid
  identity, avoiding an extra ACT table load), per-core sum(e), sum(tension),
  and sum_c e_c * o[:, c].
- Host finishes the tiny cross-core pieces: debate blend (global mean),
  softmax normalization and the final [1,1024]x[1024,512] matvec.

All large DRAM tensors are pre-tiled on host to partition-major [128, T, N]
so every DMA moves >=4KB contiguous per partition (full HBM bandwidth).
"""

import numpy as np
import ml_dtypes

import concourse.bass as bass
import concourse.mybir as mybir
import concourse.tile as tile
from concourse.tile import ScopedClock
from concourse.bass_utils import run_bass_kernel_spmd

N_CELLS = 2048
D_IN = 512
D_HID = 1024
D_OUT = 512
SYNC = 0.15
DEBATE = 0.15
N_FACTIONS = 8
N_CORES = 8
C = N_CELLS // N_CORES  # 256 cells per core

F32 = mybir.dt.float32
AF = mybir.ActivationFunctionType
ALU = mybir.AluOpType

DT_MM = mybir.dt.bfloat16  # matmul dtype
NP_MM = ml_dtypes.bfloat16 if DT_MM == mybir.dt.bfloat16 else np.float32

KH = D_HID // 128   # 8 k-tiles over D_HID
MO = D_OUT // 128   # 4 m-tiles over D_OUT


class SplitDrainTileContext(tile.TileContext):
    """TileContext whose exit drain splits sem waits across multiple drain
    instructions (this walrus build allows only one sync wait per CTRL inst)."""

    def _drain_and_barrier(self, tick_clock, wait_clock):
        nc = self.nc
        drain_inst = nc.sync.drain()
        wait_clock.add_sem_waits(
            drain_inst.ins, ScopedClock({None: tick_clock.global_clock})
        )
        si = drain_inst.ins.sync_info
        waits = list(si.on_wait) if si is not None else []
        if len(waits) > 1:
            drain_inst.ins.sync_info = mybir.SyncInfo(
                on_wait=[waits[0]], on_update=[]
            )
            for w in waits[1:]:
                d2 = nc.sync.drain()
                d2.ins.sync_info = mybir.SyncInfo(on_wait=[w], on_update=[])
        nc.all_engine_barrier()
        assert self.sems is not None
        popped = nc._tile_sem_poison_stack.pop()
        assert popped is self._sem_poison
        nc.clear_and_free_semaphores(list(self.sems.allocated().values()))
        nc.all_engine_barrier()


def _split_multi_waits(nc):
    """This walrus build allows only one sync wait per instruction; hoist
    extra waits into wait-only EventSemaphore insts on the same engine stream
    (same program point, so semantics are identical for in-order engines)."""
    n = 0
    for fn in nc.m.functions:
        for bb in fn.blocks:
            lst = bb.instructions
            new = []
            for inst in lst:
                si = inst.sync_info
                if si is not None and len(si.on_wait) > 1:
                    waits = list(si.on_wait)
                    if type(inst).__name__ == "InstDMACopy":
                        # keep the queue's own ring-backpressure wait on the
                        # descriptor; hoist data-producer waits to issue time
                        keep_i = next(
                            (i for i in range(len(waits) - 1, -1, -1)
                             if (waits[i].ant_name or "").startswith("DMAHW")),
                            len(waits) - 1,
                        )
                    else:
                        keep_i = len(waits) - 1
                    keep = waits.pop(keep_i)
                    for w in waits:
                        n += 1
                        new.append(mybir.InstEventSemaphore(
                            name=f"I-splitw-{n}", engine=inst.engine,
                            ins=[], outs=[],
                            sync_info=mybir.SyncInfo(on_wait=[w], on_update=[])))
                    inst.sync_info = mybir.SyncInfo(
                        on_wait=[keep], on_update=list(si.on_update))
                new.append(inst)
            lst[:] = new
    return n


def build_nc():
    nc = bass.Bass(trn_type="TRN2")

    # ---- DRAM tensors (pre-tiled partition-major layouts) ----
    hreT = nc.dram_tensor("hreT", [128, KH, C], F32, kind="ExternalInput")
    himT = nc.dram_tensor("himT", [128, KH, C], F32, kind="ExternalInput")
    hreTm = nc.dram_tensor("hreTm", [128, KH, C], DT_MM, kind="ExternalInput")
    himTm = nc.dram_tensor("himTm", [128, KH, C], DT_MM, kind="ExternalInput")
    hnimTm = nc.dram_tensor("hnimTm", [128, KH, C], DT_MM, kind="ExternalInput")
    wdr = nc.dram_tensor("wdr", [128, KH, D_OUT], DT_MM, kind="ExternalInput")
    wdi = nc.dram_tensor("wdi", [128, KH, D_OUT], DT_MM, kind="ExternalInput")
    wzr = nc.dram_tensor("wzr", [128, 12, 2 * D_HID], DT_MM, kind="ExternalInput")
    wzr12d = nc.dram_tensor("wzr12", [1, 2 * D_HID], DT_MM, kind="ExternalInput")
    wgr = nc.dram_tensor("wgr", [128, 12, D_HID], DT_MM, kind="ExternalInput")
    wgr12d = nc.dram_tensor("wgr12", [1, D_HID], DT_MM, kind="ExternalInput")
    wgi = nc.dram_tensor("wgi", [128, 12, D_HID], DT_MM, kind="ExternalInput")
    biasp = nc.dram_tensor("biasp", [128, 40], F32, kind="ExternalInput")

    nhreT = nc.dram_tensor("nhreT", [128, KH, C], F32, kind="ExternalOutput")
    nhimT = nc.dram_tensor("nhimT", [128, KH, C], F32, kind="ExternalOutput")
    smalls = nc.dram_tensor("smalls", [128, 16], F32, kind="ExternalOutput")

    with SplitDrainTileContext(nc) as tc:
        with (
            tc.tile_pool(name="singles", bufs=1) as singles,
            tc.tile_pool(name="hmm", bufs=3) as hmm_pool,
            tc.tile_pool(name="wg", bufs=4) as wg_pool,
            tc.tile_pool(name="cand", bufs=3) as cand_pool,
            tc.tile_pool(name="work", bufs=3) as work,
            tc.tile_pool(name="rbuf", bufs=2) as rbuf,
            tc.tile_pool(name="bigscr", bufs=2) as bigscr,
            tc.tile_pool(name="psum", bufs=8, space="PSUM") as psum,
        ):
            # ---- GEMM1-critical loads first ----
            s_hrem = hmm_pool.tile([128, KH, C], DT_MM, tag="hmm")
            nc.sync.dma_start(out=s_hrem, in_=hreTm[:, :, :])
            s_hnimm = hmm_pool.tile([128, KH, C], DT_MM, tag="hmm")
            nc.sync.dma_start(out=s_hnimm, in_=hnimTm[:, :, :])
            s_wdr = singles.tile([128, KH, D_OUT], DT_MM)
            nc.sync.dma_start(out=s_wdr, in_=wdr[:, :, :])
            s_wdi = singles.tile([128, KH, D_OUT], DT_MM)
            nc.sync.dma_start(out=s_wdi, in_=wdi[:, :, :])
            s_himm = hmm_pool.tile([128, KH, C], DT_MM, tag="hmm")
            nc.sync.dma_start(out=s_himm, in_=himTm[:, :, :])
            s_bias = singles.tile([128, 40], F32)
            nc.sync.dma_start(out=s_bias, in_=biasp[:, :])

            # then the big z/r weights and the f32 h copies
            s_wzr = singles.tile([128, 12, 2 * D_HID], DT_MM)
            nc.sync.dma_start(out=s_wzr, in_=wzr[:, :, :])
            s_wzr12 = singles.tile([1, 2 * D_HID], DT_MM)
            nc.sync.dma_start(out=s_wzr12, in_=wzr12d[:, :])
            s_hre = singles.tile([128, KH, C], F32)
            nc.sync.dma_start(out=s_hre, in_=hreT[:, :, :])
            s_him = singles.tile([128, KH, C], F32)
            nc.sync.dma_start(out=s_him, in_=himT[:, :, :])

            ones_f32 = singles.tile([128, 1], F32)
            nc.vector.memset(ones_f32, 1.0)
            ones_row = singles.tile([1, 128], DT_MM)
            nc.vector.memset(ones_row, 1.0)

            # ---- h_mag (batched halves; GEMM2-critical, on ACT during GEMM1) ----
            hmagm = singles.tile([128, KH, C], DT_MM)
            for half in range(2):
                sl = slice(half * MO, (half + 1) * MO)
                hs1 = bigscr.tile([128, MO, C], F32, tag="bigscr")
                hs2 = bigscr.tile([128, MO, C], F32, tag="bigscr")
                nc.scalar.activation(hs1, s_hre[:, sl, :], AF.Square)
                nc.scalar.activation(hs2, s_him[:, sl, :], AF.Square)
                nc.vector.tensor_add(hs1, hs1, hs2)
                nc.scalar.activation(hmagm[:, sl, :], hs1, AF.Sqrt)

            # ---- Phase A: engine GEMM -> oT [512, 256] re/im ----
            o_re = singles.tile([128, MO, C], F32)
            o_im = singles.tile([128, MO, C], F32)
            o_rem = singles.tile([128, MO, C], DT_MM)
            o_imm = singles.tile([128, MO, C], DT_MM)
            xmagm = singles.tile([128, MO, C], DT_MM)

            for m in range(MO):
                ms = slice(m * 128, (m + 1) * 128)
                p_re = psum.tile([128, C], F32, tag="ps")
                for t in range(KH):
                    nc.tensor.matmul(p_re, s_wdr[:, t, ms], s_hrem[:, t, :],
                                     start=(t == 0), stop=False)
                for t in range(KH):
                    nc.tensor.matmul(p_re, s_wdi[:, t, ms], s_hnimm[:, t, :],
                                     start=False, stop=(t == KH - 1))
                p_im = psum.tile([128, C], F32, tag="ps")
                for t in range(KH):
                    nc.tensor.matmul(p_im, s_wdr[:, t, ms], s_himm[:, t, :],
                                     start=(t == 0), stop=False)
                for t in range(KH):
                    nc.tensor.matmul(p_im, s_wdi[:, t, ms], s_hrem[:, t, :],
                                     start=False, stop=(t == KH - 1))
                # o = psum + u  (ACT Identity with per-partition bias)
                nc.scalar.activation(o_re[:, m, :], p_re, AF.Identity,
                                     bias=s_bias[:, m:m + 1])
                nc.scalar.activation(o_im[:, m, :], p_im, AF.Identity,
                                     bias=s_bias[:, 4 + m:5 + m])

            # batched epilogue: casts, sq = o_re^2 + o_im^2, x_mag = sqrt(sq)
            nc.gpsimd.tensor_copy(o_rem[:, :, :], o_re[:, :, :])
            nc.gpsimd.tensor_copy(o_imm[:, :, :], o_im[:, :, :])
            osq1 = bigscr.tile([128, MO, C], F32, tag="bigscr")
            osq2 = bigscr.tile([128, MO, C], F32, tag="bigscr")
            nc.scalar.activation(osq1, o_re[:, :, :], AF.Square)
            nc.scalar.activation(osq2, o_im[:, :, :], AF.Square)
            nc.vector.tensor_add(osq1, osq1, osq2)
            nc.scalar.activation(xmagm[:, :, :], osq1, AF.Sqrt)

            # ---- tension = mean_feat(osq1) : PE partition-sum (fp32 matmul) ----
            p_t = psum.tile([1, C], F32, tag="ps")
            for m in range(MO):
                nc.tensor.matmul(p_t, ones_f32, osq1[:, m, :],
                                 start=(m == 0), stop=(m == MO - 1))
            tension = singles.tile([1, C], F32)
            nc.scalar.activation(tension, p_t, AF.Copy, scale=1.0 / D_OUT)
            tension_m = singles.tile([1, C], DT_MM)
            nc.gpsimd.tensor_copy(tension_m, tension)

            # prefetch candidate weights (2 of 4 'wg' slots)
            wgr_a = wg_pool.tile([128, 6, D_HID], DT_MM, tag="wg")
            nc.sync.dma_start(out=wgr_a, in_=wgr[:, 0:6, :])
            wgr_b = wg_pool.tile([128, 6, D_HID], DT_MM, tag="wg")
            nc.sync.dma_start(out=wgr_b, in_=wgr[:, 6:12, :])
            s_wgr12 = singles.tile([1, D_HID], DT_MM)
            nc.sync.dma_start(out=s_wgr12, in_=wgr12d[:, :])

            # ---- Phase C: z | r GEMM over cm = [x_mag, h_mag, tension] ----
            z_f = singles.tile([128, KH, C], F32)
            rhrem = hmm_pool.tile([128, KH, C], DT_MM, tag="hmm")
            rhimm = hmm_pool.tile([128, KH, C], DT_MM, tag="hmm")
            # consume h_mag k-tiles first: x_mag (phase-A epilogue) lands later
            t_order = list(range(4, 12)) + list(range(0, 4))
            for m in range(16):
                ms = slice(m * 128, (m + 1) * 128)
                p = psum.tile([128, C], F32, tag="ps")
                for i, t in enumerate(t_order):
                    rhs = xmagm[:, t, :] if t < 4 else hmagm[:, t - 4, :]
                    nc.tensor.matmul(p, s_wzr[:, t, ms], rhs,
                                     start=(i == 0), stop=False)
                nc.tensor.matmul(p, s_wzr12[:, ms], tension_m,
                                 start=False, stop=True)
                if m < 8:
                    nc.scalar.activation(z_f[:, m, :], p, AF.Sigmoid,
                                         bias=s_bias[:, 8 + m:9 + m])
                else:
                    j = m - 8
                    r_t = rbuf.tile([128, C], F32, tag="rbuf")
                    nc.scalar.activation(r_t, p, AF.Sigmoid,
                                         bias=s_bias[:, 16 + j:17 + j])
                    # r*h in matmul dtype, immediately after r is ready
                    nc.vector.tensor_mul(rhrem[:, j, :], r_t, s_hre[:, j, :])
                    nc.vector.tensor_mul(rhimm[:, j, :], r_t, s_him[:, j, :])

            # ---- softmax partials (overlap with candidate GEMMs) ----
            # e = exp(tension) = s/(1-s), s = sigmoid(tension)
            sig = work.tile([1, C], F32, tag="tiny")
            om = work.tile([1, C], F32, tag="tiny")
            e_f = singles.tile([1, C], F32)
            nc.scalar.activation(sig, tension, AF.Sigmoid)
            nc.vector.tensor_scalar(om, sig, -1.0, 1.0, op0=ALU.mult, op1=ALU.add)
            nc.vector.reciprocal(om, om)
            nc.vector.tensor_mul(e_f, sig, om)
            e_m = singles.tile([1, C], DT_MM)
            nc.gpsimd.tensor_copy(e_m, e_f)
            p_eb = psum.tile([128, C], F32, tag="ps")
            nc.tensor.matmul(p_eb, ones_row, e_m, start=True, stop=True)
            e_b = singles.tile([128, C], F32)
            nc.scalar.copy(e_b, p_eb)

            s_small = singles.tile([128, 16], F32)
            nc.vector.memset(s_small, 0.0)
            e_b_ap = e_b[:, :]
            e_b4 = bass.AP(tensor=e_b_ap.tensor, offset=e_b_ap.offset,
                           ap=[e_b_ap.ap[0], [0, MO], e_b_ap.ap[1]])
            wsum = bigscr.tile([128, MO, C], F32, tag="bigscr")
            nc.vector.tensor_mul(wsum, o_re[:, :, :], e_b4)
            nc.vector.reduce_sum(s_small[:, 0:MO], wsum,
                                 axis=mybir.AxisListType.X)
            wsum2 = bigscr.tile([128, MO, C], F32, tag="bigscr")
            nc.vector.tensor_mul(wsum2, o_im[:, :, :], e_b4)
            nc.vector.reduce_sum(s_small[:, MO:2 * MO], wsum2,
                                 axis=mybir.AxisListType.X)
            nc.vector.reduce_sum(s_small[0:1, 8:9], e_f,
                                 axis=mybir.AxisListType.X)
            nc.vector.reduce_sum(s_small[0:1, 9:10], tension,
                                 axis=mybir.AxisListType.X)

            # ---- Phase E+F: candidate GEMMs fused with the GRU/sync update ----
            fscale = singles.tile([128, 2 * KH], F32)

            def nh_update(comp, j, cand_t):
                h_t = s_hre if comp == 0 else s_him
                out_d = nhreT if comp == 0 else nhimT
                scr = work.tile([128, C], F32, tag="scr")
                nh = work.tile([128, C], F32, tag="nh")
                nc.vector.tensor_sub(scr, cand_t, h_t[:, j, :])
                nc.vector.tensor_mul(scr, z_f[:, j, :], scr)
                nc.vector.tensor_add(nh, h_t[:, j, :], scr)
                col = fscale[:, comp * KH + j:comp * KH + j + 1]
                nc.vector.reduce_sum(col, nh, axis=mybir.AxisListType.X)
                nc.vector.tensor_scalar_mul(col, col, SYNC / C)
                nc.vector.tensor_scalar(nh, nh, 1.0 - SYNC, col,
                                        op0=ALU.mult, op1=ALU.add)
                nc.sync.dma_start(out=out_d[:, j, :], in_=nh)

            wgi_tiles = []
            for m in range(KH):
                ms = slice(m * 128, (m + 1) * 128)
                p = psum.tile([128, C], F32, tag="ps")
                for t in range(12):
                    wt = wgr_a[:, t, ms] if t < 6 else wgr_b[:, t - 6, ms]
                    rhs = o_rem[:, t, :] if t < 4 else rhrem[:, t - 4, :]
                    nc.tensor.matmul(p, wt, rhs, start=(t == 0), stop=False)
                nc.tensor.matmul(p, s_wgr12[:, ms], tension_m,
                                 start=False, stop=True)
                cand_t = cand_pool.tile([128, C], F32, tag="cand")
                nc.scalar.activation(cand_t, p, AF.Tanh,
                                     bias=s_bias[:, 24 + m:25 + m])
                nh_update(0, m, cand_t)
                if m == 0:
                    # prefetch wgi into the two spare 'wg' slots
                    wgi_a = wg_pool.tile([128, 6, D_HID], DT_MM, tag="wg")
                    nc.sync.dma_start(out=wgi_a, in_=wgi[:, 0:6, :])
                    wgi_tiles.append(wgi_a)
                if m == 1:
                    wgi_b = wg_pool.tile([128, 6, D_HID], DT_MM, tag="wg")
                    nc.sync.dma_start(out=wgi_b, in_=wgi[:, 6:12, :])
                    wgi_tiles.append(wgi_b)

            for m in range(KH):
                ms = slice(m * 128, (m + 1) * 128)
                p = psum.tile([128, C], F32, tag="ps")
                for t in range(12):
                    wt = wgi_tiles[0][:, t, ms] if t < 6 else wgi_tiles[1][:, t - 6, ms]
                    rhs = o_imm[:, t, :] if t < 4 else rhimm[:, t - 4, :]
                    nc.tensor.matmul(p, wt, rhs, start=(t == 0), stop=(t == 11))
                cand_t = cand_pool.tile([128, C], F32, tag="cand")
                nc.scalar.activation(cand_t, p, AF.Tanh,
                                     bias=s_bias[:, 32 + m:33 + m])
                nh_update(1, m, cand_t)

            nc.sync.dma_start(out=smalls[:, :], in_=s_small)

    _split_multi_waits(nc)
    return nc


_NC_CACHE = {}


def _get_nc():
    if "nc" not in _NC_CACHE:
        _NC_CACHE["nc"] = build_nc()
    return _NC_CACHE["nc"]


def _tile8(a, np_dt):
    """[T*128, N] row-major -> partition-major [128, T, N] contiguous."""
    t = a.shape[0] // 128
    return np.ascontiguousarray(
        a.reshape(t, 128, a.shape[1]).transpose(1, 0, 2)
    ).astype(np_dt, copy=False)


def _prep_in_maps(inputs):
    f = {k: np.asarray(v, dtype=np.float32)
         for k, v in inputs.items() if k != "step"}
    x = f["x"]
    h_re = f["h_re"]
    h_im = f["h_im"]

    Dr = f["ea_wr"] - f["eg_wr"]
    Di = f["ea_wi"] - f["eg_wi"]
    dbr = f["ea_br"] - f["eg_br"]
    dbi = f["ea_bi"] - f["eg_bi"]
    u_re = (x @ Dr[:D_IN])[0] + dbr - dbi
    u_im = (x @ Di[:D_IN])[0] + dbr + dbi
    wdr = _tile8(Dr[D_IN:].astype(NP_MM), NP_MM)
    wdi = _tile8(Di[D_IN:].astype(NP_MM), NP_MM)

    def reorder(w):
        return np.concatenate([w[0:512], w[513:1537], w[512:513]], axis=0)

    wzr_full = reorder(np.concatenate([f["gz_w"], f["gr_w"]], axis=1)).astype(NP_MM)
    wgr_full = reorder(f["ghr_w"]).astype(NP_MM)
    wgi_full = np.concatenate(
        [f["ghi_w"][0:512], f["ghi_w"][513:1537]], axis=0).astype(NP_MM)

    biasp = np.zeros((128, 40), np.float32)
    for vec, c0 in [
        (u_re, 0), (u_im, 4),
        (f["gz_b"], 8), (f["gr_b"], 16), (f["ghr_b"], 24), (f["ghi_b"], 32),
    ]:
        nch = vec.shape[0] // 128
        biasp[:, c0:c0 + nch] = vec.reshape(nch, 128).T

    shared = dict(
        wdr=wdr, wdi=wdi,
        wzr=_tile8(wzr_full[0:1536], NP_MM),
        wzr12=np.ascontiguousarray(wzr_full[1536:1537]),
        wgr=_tile8(wgr_full[0:1536], NP_MM),
        wgr12=np.ascontiguousarray(wgr_full[1536:1537]),
        wgi=_tile8(wgi_full, NP_MM),
        biasp=biasp,
    )
    in_maps = []
    for c in range(N_CORES):
        hrT = np.ascontiguousarray(h_re[c * C:(c + 1) * C].T)  # [1024, 256]
        hiT = np.ascontiguousarray(h_im[c * C:(c + 1) * C].T)
        in_maps.append(dict(
            shared,
            hreT=_tile8(hrT, np.float32), himT=_tile8(hiT, np.float32),
            hreTm=_tile8(hrT.astype(NP_MM), NP_MM),
            himTm=_tile8(hiT.astype(NP_MM), NP_MM),
            hnimTm=_tile8((-hiT).astype(NP_MM), NP_MM),
        ))
    return in_maps


def _untile8(a):
    """[128, T, N] -> [T*128, N]."""
    return a.transpose(1, 0, 2).reshape(-1, a.shape[2])


def _assemble(inputs, results):
    step = int(np.asarray(inputs["step"]))
    oh_w = np.asarray(inputs["oh_w"], np.float32)
    oh_b = np.asarray(inputs["oh_b"], np.float32)

    nh_re = np.concatenate([_untile8(r["nhreT"]).T for r in results], axis=0)
    nh_im = np.concatenate([_untile8(r["nhimT"]).T for r in results], axis=0)
    if step > 5:
        dc = max(1, C // 4)
        for nh in (nh_re, nh_im):
            glob = nh.mean(axis=0, dtype=np.float32).astype(np.float32)
            for fct in range(N_FACTIONS):
                rows = slice(fct * C, fct * C + dc)
                nh[rows] = (1.0 - DEBATE) * nh[rows] + DEBATE * glob

    comb_re = np.zeros(512, np.float32)
    comb_im = np.zeros(512, np.float32)
    expsum = np.float32(0.0)
    tsum = np.float32(0.0)
    for r in results:
        s = r["smalls"]
        comb_re += s[:, 0:4].T.reshape(-1)
        comb_im += s[:, 4:8].T.reshape(-1)
        expsum += s[0, 8]
        tsum += s[0, 9]
    comb = np.concatenate([comb_re, comb_im]) / expsum
    pred = (comb[None, :] @ oh_w + oh_b).astype(np.float32)
    mt = np.float32(tsum / N_CELLS)
    return pred, mt, np.ascontiguousarray(nh_re, dtype=np.float32), \
        np.ascontiguousarray(nh_im, dtype=np.float32)


def _kernel_impl(inputs, trace=False, tmpdir=None):
    nc = _get_nc()
    in_maps = _prep_in_maps(inputs)
    res = run_bass_kernel_spmd(nc, in_maps, core_ids=list(range(N_CORES)),
                               trace=trace, tmpdir=tmpdir)
    return _assemble(inputs, res.results), res


def kernel(**inputs):
    outs, _ = _kernel_impl(inputs)
    return outs


# revision 15
# speedup vs baseline: 1.1096x; 1.1096x over previous
"""Trainium2 Bass kernel for nn_ComplexValuedEngine.

Strategy (8 NeuronCores, data-parallel over cells):
- 2048 cells / 8 cores = 256 cells per core = exactly one faction per core,
  so the faction-sync mean is a purely local free-dim reduction.
- All activations live TRANSPOSED on device: [features, cells]. Every GEMM is
  then out[m_feat_tile, cells] = W_slice.T @ actT with natural-layout weights
  as the PE stationary operand and N=256 moving (full-rate bf16).
- Host folds the two complex linears (o = a - g  =>  single weight diff Dr/Di)
  and turns the shared x-row into per-feature biases u_re/u_im.
- GRU inputs are reordered so cm = [x_mag(512), h_mag(1024), tension(1)]:
  12 clean K=128 tiles plus one K=1 rank-1 matmul for the tension feature.
- Device also computes softmax partials: e = exp(tension) (via the sigmoid
  identity, avoiding an extra ACT table load), per-core sum(e), sum(tension),
  and sum_c e_c * o[:, c].
- Host finishes the tiny cross-core pieces: debate blend (global mean),
  softmax normalization and the final [1,1024]x[1024,512] matvec.

All large DRAM tensors are pre-tiled on host to partition-major [128, T, N]
so every DMA moves >=4KB contiguous per partition (full HBM bandwidth).
"""

import numpy as np
import ml_dtypes

import concourse.bass as bass
import concourse.mybir as mybir
import concourse.tile as tile
from concourse.tile import ScopedClock
from concourse.bass_utils import run_bass_kernel_spmd

N_CELLS = 2048
D_IN = 512
D_HID = 1024
D_OUT = 512
SYNC = 0.15
DEBATE = 0.15
N_FACTIONS = 8
N_CORES = 8
C = N_CELLS // N_CORES  # 256 cells per core

F32 = mybir.dt.float32
AF = mybir.ActivationFunctionType
ALU = mybir.AluOpType

DT_MM = mybir.dt.bfloat16  # matmul dtype
NP_MM = ml_dtypes.bfloat16 if DT_MM == mybir.dt.bfloat16 else np.float32

KH = D_HID // 128   # 8 k-tiles over D_HID
MO = D_OUT // 128   # 4 m-tiles over D_OUT


class SplitDrainTileContext(tile.TileContext):
    """TileContext whose exit drain splits sem waits across multiple drain
    instructions (this walrus build allows only one sync wait per CTRL inst)."""

    def _drain_and_barrier(self, tick_clock, wait_clock):
        nc = self.nc
        drain_inst = nc.sync.drain()
        wait_clock.add_sem_waits(
            drain_inst.ins, ScopedClock({None: tick_clock.global_clock})
        )
        si = drain_inst.ins.sync_info
        waits = list(si.on_wait) if si is not None else []
        if len(waits) > 1:
            drain_inst.ins.sync_info = mybir.SyncInfo(
                on_wait=[waits[0]], on_update=[]
            )
            for w in waits[1:]:
                d2 = nc.sync.drain()
                d2.ins.sync_info = mybir.SyncInfo(on_wait=[w], on_update=[])
        nc.all_engine_barrier()
        assert self.sems is not None
        popped = nc._tile_sem_poison_stack.pop()
        assert popped is self._sem_poison
        nc.clear_and_free_semaphores(list(self.sems.allocated().values()))
        nc.all_engine_barrier()


def _split_multi_waits(nc):
    """This walrus build allows only one sync wait per instruction; hoist
    extra waits into wait-only EventSemaphore insts on the same engine stream
    (same program point, so semantics are identical for in-order engines)."""
    n = 0
    for fn in nc.m.functions:
        for bb in fn.blocks:
            lst = bb.instructions
            new = []
            for inst in lst:
                si = inst.sync_info
                if si is not None and len(si.on_wait) > 1:
                    waits = list(si.on_wait)
                    if type(inst).__name__ == "InstDMACopy":
                        # keep the queue's own ring-backpressure wait on the
                        # descriptor; hoist data-producer waits to issue time
                        keep_i = next(
                            (i for i in range(len(waits) - 1, -1, -1)
                             if (waits[i].ant_name or "").startswith("DMAHW")),
                            len(waits) - 1,
                        )
                    else:
                        keep_i = len(waits) - 1
                    keep = waits.pop(keep_i)
                    for w in waits:
                        n += 1
                        new.append(mybir.InstEventSemaphore(
                            name=f"I-splitw-{n}", engine=inst.engine,
                            ins=[], outs=[],
                            sync_info=mybir.SyncInfo(on_wait=[w], on_update=[])))
                    inst.sync_info = mybir.SyncInfo(
                        on_wait=[keep], on_update=list(si.on_update))
                new.append(inst)
            lst[:] = new
    return n


def build_nc():
    nc = bass.Bass(trn_type="TRN2")

    # ---- DRAM tensors (pre-tiled partition-major layouts) ----
    hreT = nc.dram_tensor("hreT", [128, KH, C], F32, kind="ExternalInput")
    himT = nc.dram_tensor("himT", [128, KH, C], F32, kind="ExternalInput")
    wdr = nc.dram_tensor("wdr", [128, KH, D_OUT], DT_MM, kind="ExternalInput")
    wdi = nc.dram_tensor("wdi", [128, KH, D_OUT], DT_MM, kind="ExternalInput")
    wzr = nc.dram_tensor("wzr", [128, 12, 2 * D_HID], DT_MM, kind="ExternalInput")
    wzr12d = nc.dram_tensor("wzr12", [1, 2 * D_HID], DT_MM, kind="ExternalInput")
    wgr = nc.dram_tensor("wgr", [128, 12, D_HID], DT_MM, kind="ExternalInput")
    wgr12d = nc.dram_tensor("wgr12", [1, D_HID], DT_MM, kind="ExternalInput")
    wgi = nc.dram_tensor("wgi", [128, 12, D_HID], DT_MM, kind="ExternalInput")
    biasp = nc.dram_tensor("biasp", [128, 40], F32, kind="ExternalInput")

    nhreT = nc.dram_tensor("nhreT", [128, KH, C], F32, kind="ExternalOutput")
    nhimT = nc.dram_tensor("nhimT", [128, KH, C], F32, kind="ExternalOutput")
    smalls = nc.dram_tensor("smalls", [128, 16], F32, kind="ExternalOutput")

    with SplitDrainTileContext(nc) as tc:
        with (
            tc.tile_pool(name="singles", bufs=1) as singles,
            tc.tile_pool(name="hmm", bufs=3) as hmm_pool,
            tc.tile_pool(name="wg", bufs=4) as wg_pool,
            tc.tile_pool(name="cand", bufs=3) as cand_pool,
            tc.tile_pool(name="work", bufs=3) as work,
            tc.tile_pool(name="rbuf", bufs=2) as rbuf,
            tc.tile_pool(name="bigscr", bufs=2) as bigscr,
            tc.tile_pool(name="psum", bufs=8, space="PSUM") as psum,
        ):
            # ---- GEMM1-critical loads first (f32 h; bf16 copies cast on-device) ----
            s_hre = singles.tile([128, KH, C], F32)
            nc.sync.dma_start(out=s_hre, in_=hreT[:, :, :])
            s_hrem = hmm_pool.tile([128, KH, C], DT_MM, tag="hmm")
            nc.vector.tensor_copy(s_hrem, s_hre[:, :, :])
            s_wdr = singles.tile([128, KH, D_OUT], DT_MM)
            nc.sync.dma_start(out=s_wdr[:, 0:4, :], in_=wdr[:, 0:4, :])
            nc.sync.dma_start(out=s_wdr[:, 4:KH, :], in_=wdr[:, 4:KH, :])
            s_him = singles.tile([128, KH, C], F32)
            nc.sync.dma_start(out=s_him, in_=himT[:, :, :])
            s_hnimm = hmm_pool.tile([128, KH, C], DT_MM, tag="hmm")
            nc.vector.tensor_scalar_mul(s_hnimm, s_him[:, :, :], -1.0)
            s_himm = hmm_pool.tile([128, KH, C], DT_MM, tag="hmm")
            nc.gpsimd.tensor_copy(s_himm, s_him[:, :, :])
            s_wdi = singles.tile([128, KH, D_OUT], DT_MM)
            nc.sync.dma_start(out=s_wdi[:, 0:4, :], in_=wdi[:, 0:4, :])
            nc.sync.dma_start(out=s_wdi[:, 4:KH, :], in_=wdi[:, 4:KH, :])
            s_bias = singles.tile([128, 40], F32)
            nc.sync.dma_start(out=s_bias, in_=biasp[:, :])

            # z/r weights, chunked in GEMM2 consumption order (h_mag tiles first)
            s_wzr = singles.tile([128, 12, 2 * D_HID], DT_MM)
            nc.sync.dma_start(out=s_wzr[:, 4:8, :], in_=wzr[:, 4:8, :])
            nc.sync.dma_start(out=s_wzr[:, 8:12, :], in_=wzr[:, 8:12, :])
            nc.sync.dma_start(out=s_wzr[:, 0:4, :], in_=wzr[:, 0:4, :])
            s_wzr12 = singles.tile([1, 2 * D_HID], DT_MM)
            nc.sync.dma_start(out=s_wzr12, in_=wzr12d[:, :])

            ones_f32 = singles.tile([128, 1], F32)
            nc.vector.memset(ones_f32, 1.0)
            ones_row = singles.tile([1, 128], DT_MM)
            nc.vector.memset(ones_row, 1.0)

            # ---- h_mag squares on DVE (idle during GEMM1); sqrt on ACT later ----
            hmagm = singles.tile([128, KH, C], DT_MM)
            hm_halves = []
            for half in range(2):
                sl = slice(half * MO, (half + 1) * MO)
                hs1 = bigscr.tile([128, MO, C], F32, tag="bigscr")
                hs2 = bigscr.tile([128, MO, C], F32, tag="bigscr")
                nc.vector.tensor_mul(hs1, s_hre[:, sl, :], s_hre[:, sl, :])
                nc.vector.tensor_mul(hs2, s_him[:, sl, :], s_him[:, sl, :])
                nc.vector.tensor_add(hs1, hs1, hs2)
                hm_halves.append((sl, hs1))

            # ---- Phase A: engine GEMM -> oT [512, 256] re/im ----
            o_re = singles.tile([128, MO, C], F32)
            o_im = singles.tile([128, MO, C], F32)
            o_rem = singles.tile([128, MO, C], DT_MM)
            o_imm = singles.tile([128, MO, C], DT_MM)
            xmagm = singles.tile([128, MO, C], DT_MM)

            for m in range(MO):
                ms = slice(m * 128, (m + 1) * 128)
                p_re = psum.tile([128, C], F32, tag="ps")
                for t in range(KH):
                    nc.tensor.matmul(p_re, s_wdr[:, t, ms], s_hrem[:, t, :],
                                     start=(t == 0), stop=False)
                for t in range(KH):
                    nc.tensor.matmul(p_re, s_wdi[:, t, ms], s_hnimm[:, t, :],
                                     start=False, stop=(t == KH - 1))
                p_im = psum.tile([128, C], F32, tag="ps")
                for t in range(KH):
                    nc.tensor.matmul(p_im, s_wdr[:, t, ms], s_himm[:, t, :],
                                     start=(t == 0), stop=False)
                for t in range(KH):
                    nc.tensor.matmul(p_im, s_wdi[:, t, ms], s_hrem[:, t, :],
                                     start=False, stop=(t == KH - 1))
                # o = psum + u  (ACT Identity with per-partition bias)
                nc.scalar.activation(o_re[:, m, :], p_re, AF.Identity,
                                     bias=s_bias[:, m:m + 1])
                nc.scalar.activation(o_im[:, m, :], p_im, AF.Identity,
                                     bias=s_bias[:, 4 + m:5 + m])

            # h_mag sqrts (ACT, directly after the GEMM1 psum drains)
            for sl, hs1 in hm_halves:
                nc.scalar.activation(hmagm[:, sl, :], hs1, AF.Sqrt)

            # batched epilogue: casts, sq = o_re^2 + o_im^2, x_mag = sqrt(sq)
            nc.gpsimd.tensor_copy(o_rem[:, :, :], o_re[:, :, :])
            nc.gpsimd.tensor_copy(o_imm[:, :, :], o_im[:, :, :])
            osq1 = bigscr.tile([128, MO, C], F32, tag="bigscr")
            osq2 = bigscr.tile([128, MO, C], F32, tag="bigscr")
            nc.scalar.activation(osq1, o_re[:, :, :], AF.Square)
            nc.scalar.activation(osq2, o_im[:, :, :], AF.Square)
            nc.vector.tensor_add(osq1, osq1, osq2)
            nc.scalar.activation(xmagm[:, :, :], osq1, AF.Sqrt)

            # ---- tension = mean_feat(osq1) : PE partition-sum (fp32 matmul) ----
            p_t = psum.tile([1, C], F32, tag="ps")
            for m in range(MO):
                nc.tensor.matmul(p_t, ones_f32, osq1[:, m, :],
                                 start=(m == 0), stop=(m == MO - 1))
            tension = singles.tile([1, C], F32)
            nc.scalar.activation(tension, p_t, AF.Copy, scale=1.0 / D_OUT)
            tension_m = singles.tile([1, C], DT_MM)
            nc.gpsimd.tensor_copy(tension_m, tension)

            # prefetch candidate weights (2 of 4 'wg' slots)
            wgr_a = wg_pool.tile([128, 6, D_HID], DT_MM, tag="wg")
            nc.sync.dma_start(out=wgr_a, in_=wgr[:, 0:6, :])
            wgr_b = wg_pool.tile([128, 6, D_HID], DT_MM, tag="wg")
            nc.sync.dma_start(out=wgr_b, in_=wgr[:, 6:12, :])
            s_wgr12 = singles.tile([1, D_HID], DT_MM)
            nc.sync.dma_start(out=s_wgr12, in_=wgr12d[:, :])

            # ---- Phase C: z | r GEMM over cm = [x_mag, h_mag, tension] ----
            z_f = singles.tile([128, KH, C], F32)
            rhrem = hmm_pool.tile([128, KH, C], DT_MM, tag="hmm")
            rhimm = hmm_pool.tile([128, KH, C], DT_MM, tag="hmm")
            # consume h_mag k-tiles first: x_mag (phase-A epilogue) lands later
            t_order = list(range(4, 12)) + list(range(0, 4))
            for m in range(16):
                ms = slice(m * 128, (m + 1) * 128)
                p = psum.tile([128, C], F32, tag="ps")
                for i, t in enumerate(t_order):
                    rhs = xmagm[:, t, :] if t < 4 else hmagm[:, t - 4, :]
                    nc.tensor.matmul(p, s_wzr[:, t, ms], rhs,
                                     start=(i == 0), stop=False)
                nc.tensor.matmul(p, s_wzr12[:, ms], tension_m,
                                 start=False, stop=True)
                if m < 8:
                    nc.scalar.activation(z_f[:, m, :], p, AF.Sigmoid,
                                         bias=s_bias[:, 8 + m:9 + m])
                else:
                    j = m - 8
                    r_t = rbuf.tile([128, C], F32, tag="rbuf")
                    nc.scalar.activation(r_t, p, AF.Sigmoid,
                                         bias=s_bias[:, 16 + j:17 + j])
                    # r*h in matmul dtype, immediately after r is ready
                    nc.vector.tensor_mul(rhrem[:, j, :], r_t, s_hre[:, j, :])
                    nc.vector.tensor_mul(rhimm[:, j, :], r_t, s_him[:, j, :])

            # ---- softmax partials (overlap with candidate GEMMs) ----
            # e = exp(tension) = s/(1-s), s = sigmoid(tension)
            sig = work.tile([1, C], F32, tag="tiny")
            om = work.tile([1, C], F32, tag="tiny")
            e_f = singles.tile([1, C], F32)
            nc.scalar.activation(sig, tension, AF.Sigmoid)
            nc.vector.tensor_scalar(om, sig, -1.0, 1.0, op0=ALU.mult, op1=ALU.add)
            nc.vector.reciprocal(om, om)
            nc.vector.tensor_mul(e_f, sig, om)
            e_m = singles.tile([1, C], DT_MM)
            nc.gpsimd.tensor_copy(e_m, e_f)
            p_eb = psum.tile([128, C], F32, tag="ps")
            nc.tensor.matmul(p_eb, ones_row, e_m, start=True, stop=True)
            e_b = singles.tile([128, C], F32)
            nc.scalar.copy(e_b, p_eb)

            s_small = singles.tile([128, 16], F32)
            nc.vector.memset(s_small, 0.0)
            e_b_ap = e_b[:, :]
            e_b4 = bass.AP(tensor=e_b_ap.tensor, offset=e_b_ap.offset,
                           ap=[e_b_ap.ap[0], [0, MO], e_b_ap.ap[1]])
            wsum = bigscr.tile([128, MO, C], F32, tag="bigscr")
            nc.vector.tensor_mul(wsum, o_re[:, :, :], e_b4)
            nc.vector.reduce_sum(s_small[:, 0:MO], wsum,
                                 axis=mybir.AxisListType.X)
            wsum2 = bigscr.tile([128, MO, C], F32, tag="bigscr")
            nc.vector.tensor_mul(wsum2, o_im[:, :, :], e_b4)
            nc.vector.reduce_sum(s_small[:, MO:2 * MO], wsum2,
                                 axis=mybir.AxisListType.X)
            nc.vector.reduce_sum(s_small[0:1, 8:9], e_f,
                                 axis=mybir.AxisListType.X)
            nc.vector.reduce_sum(s_small[0:1, 9:10], tension,
                                 axis=mybir.AxisListType.X)

            # ---- Phase E+F: candidate GEMMs fused with the GRU/sync update ----
            fscale = singles.tile([128, 2 * KH], F32)

            def nh_update(comp, j, cand_t):
                h_t = s_hre if comp == 0 else s_him
                out_d = nhreT if comp == 0 else nhimT
                scr = work.tile([128, C], F32, tag="scr")
                nh = work.tile([128, C], F32, tag="nh")
                nc.vector.tensor_sub(scr, cand_t, h_t[:, j, :])
                nc.vector.tensor_mul(scr, z_f[:, j, :], scr)
                nc.vector.tensor_add(nh, h_t[:, j, :], scr)
                col = fscale[:, comp * KH + j:comp * KH + j + 1]
                nc.vector.reduce_sum(col, nh, axis=mybir.AxisListType.X)
                nc.vector.tensor_scalar_mul(col, col, SYNC / C)
                nc.vector.tensor_scalar(nh, nh, 1.0 - SYNC, col,
                                        op0=ALU.mult, op1=ALU.add)
                nc.sync.dma_start(out=out_d[:, j, :], in_=nh)

            wgi_tiles = []
            for m in range(KH):
                ms = slice(m * 128, (m + 1) * 128)
                p = psum.tile([128, C], F32, tag="ps")
                for t in range(12):
                    wt = wgr_a[:, t, ms] if t < 6 else wgr_b[:, t - 6, ms]
                    rhs = o_rem[:, t, :] if t < 4 else rhrem[:, t - 4, :]
                    nc.tensor.matmul(p, wt, rhs, start=(t == 0), stop=False)
                nc.tensor.matmul(p, s_wgr12[:, ms], tension_m,
                                 start=False, stop=True)
                cand_t = cand_pool.tile([128, C], F32, tag="cand")
                nc.scalar.activation(cand_t, p, AF.Tanh,
                                     bias=s_bias[:, 24 + m:25 + m])
                nh_update(0, m, cand_t)
                if m == 0:
                    # prefetch wgi into the two spare 'wg' slots
                    wgi_a = wg_pool.tile([128, 6, D_HID], DT_MM, tag="wg")
                    nc.sync.dma_start(out=wgi_a, in_=wgi[:, 0:6, :])
                    wgi_tiles.append(wgi_a)
                if m == 1:
                    wgi_b = wg_pool.tile([128, 6, D_HID], DT_MM, tag="wg")
                    nc.sync.dma_start(out=wgi_b, in_=wgi[:, 6:12, :])
                    wgi_tiles.append(wgi_b)

            for m in range(KH):
                ms = slice(m * 128, (m + 1) * 128)
                p = psum.tile([128, C], F32, tag="ps")
                for t in range(12):
                    wt = wgi_tiles[0][:, t, ms] if t < 6 else wgi_tiles[1][:, t - 6, ms]
                    rhs = o_imm[:, t, :] if t < 4 else rhimm[:, t - 4, :]
                    nc.tensor.matmul(p, wt, rhs, start=(t == 0), stop=(t == 11))
                cand_t = cand_pool.tile([128, C], F32, tag="cand")
                nc.scalar.activation(cand_t, p, AF.Tanh,
                                     bias=s_bias[:, 32 + m:33 + m])
                nh_update(1, m, cand_t)

            nc.sync.dma_start(out=smalls[:, :], in_=s_small)

    _split_multi_waits(nc)
    return nc


_NC_CACHE = {}


def _get_nc():
    if "nc" not in _NC_CACHE:
        _NC_CACHE["nc"] = build_nc()
    return _NC_CACHE["nc"]


def _tile8(a, np_dt):
    """[T*128, N] row-major -> partition-major [128, T, N] contiguous."""
    t = a.shape[0] // 128
    return np.ascontiguousarray(
        a.reshape(t, 128, a.shape[1]).transpose(1, 0, 2)
    ).astype(np_dt, copy=False)


def _prep_in_maps(inputs):
    f = {k: np.asarray(v, dtype=np.float32)
         for k, v in inputs.items() if k != "step"}
    x = f["x"]
    h_re = f["h_re"]
    h_im = f["h_im"]

    Dr = f["ea_wr"] - f["eg_wr"]
    Di = f["ea_wi"] - f["eg_wi"]
    dbr = f["ea_br"] - f["eg_br"]
    dbi = f["ea_bi"] - f["eg_bi"]
    u_re = (x @ Dr[:D_IN])[0] + dbr - dbi
    u_im = (x @ Di[:D_IN])[0] + dbr + dbi
    wdr = _tile8(Dr[D_IN:].astype(NP_MM), NP_MM)
    wdi = _tile8(Di[D_IN:].astype(NP_MM), NP_MM)

    def reorder(w):
        return np.concatenate([w[0:512], w[513:1537], w[512:513]], axis=0)

    wzr_full = reorder(np.concatenate([f["gz_w"], f["gr_w"]], axis=1)).astype(NP_MM)
    wgr_full = reorder(f["ghr_w"]).astype(NP_MM)
    wgi_full = np.concatenate(
        [f["ghi_w"][0:512], f["ghi_w"][513:1537]], axis=0).astype(NP_MM)

    biasp = np.zeros((128, 40), np.float32)
    for vec, c0 in [
        (u_re, 0), (u_im, 4),
        (f["gz_b"], 8), (f["gr_b"], 16), (f["ghr_b"], 24), (f["ghi_b"], 32),
    ]:
        nch = vec.shape[0] // 128
        biasp[:, c0:c0 + nch] = vec.reshape(nch, 128).T

    shared = dict(
        wdr=wdr, wdi=wdi,
        wzr=_tile8(wzr_full[0:1536], NP_MM),
        wzr12=np.ascontiguousarray(wzr_full[1536:1537]),
        wgr=_tile8(wgr_full[0:1536], NP_MM),
        wgr12=np.ascontiguousarray(wgr_full[1536:1537]),
        wgi=_tile8(wgi_full, NP_MM),
        biasp=biasp,
    )
    in_maps = []
    for c in range(N_CORES):
        hrT = np.ascontiguousarray(h_re[c * C:(c + 1) * C].T)  # [1024, 256]
        hiT = np.ascontiguousarray(h_im[c * C:(c + 1) * C].T)
        in_maps.append(dict(
            shared,
            hreT=_tile8(hrT, np.float32), himT=_tile8(hiT, np.float32),
        ))
    return in_maps


def _untile8(a):
    """[128, T, N] -> [T*128, N]."""
    return a.transpose(1, 0, 2).reshape(-1, a.shape[2])


def _assemble(inputs, results):
    step = int(np.asarray(inputs["step"]))
    oh_w = np.asarray(inputs["oh_w"], np.float32)
    oh_b = np.asarray(inputs["oh_b"], np.float32)

    nh_re = np.concatenate([_untile8(r["nhreT"]).T for r in results], axis=0)
    nh_im = np.concatenate([_untile8(r["nhimT"]).T for r in results], axis=0)
    if step > 5:
        dc = max(1, C // 4)
        for nh in (nh_re, nh_im):
            glob = nh.mean(axis=0, dtype=np.float32).astype(np.float32)
            for fct in range(N_FACTIONS):
                rows = slice(fct * C, fct * C + dc)
                nh[rows] = (1.0 - DEBATE) * nh[rows] + DEBATE * glob

    comb_re = np.zeros(512, np.float32)
    comb_im = np.zeros(512, np.float32)
    expsum = np.float32(0.0)
    tsum = np.float32(0.0)
    for r in results:
        s = r["smalls"]
        comb_re += s[:, 0:4].T.reshape(-1)
        comb_im += s[:, 4:8].T.reshape(-1)
        expsum += s[0, 8]
        tsum += s[0, 9]
    comb = np.concatenate([comb_re, comb_im]) / expsum
    pred = (comb[None, :] @ oh_w + oh_b).astype(np.float32)
    mt = np.float32(tsum / N_CELLS)
    return pred, mt, np.ascontiguousarray(nh_re, dtype=np.float32), \
        np.ascontiguousarray(nh_im, dtype=np.float32)


def _kernel_impl(inputs, trace=False, tmpdir=None):
    nc = _get_nc()
    in_maps = _prep_in_maps(inputs)
    res = run_bass_kernel_spmd(nc, in_maps, core_ids=list(range(N_CORES)),
                               trace=trace, tmpdir=tmpdir)
    return _assemble(inputs, res.results), res


def kernel(**inputs):
    outs, _ = _kernel_impl(inputs)
    return outs


# revision 16
# speedup vs baseline: 1.2244x; 1.1034x over previous
"""Trainium2 Bass kernel for nn_ComplexValuedEngine.

Strategy (8 NeuronCores, data-parallel over cells):
- 2048 cells / 8 cores = 256 cells per core = exactly one faction per core,
  so the faction-sync mean is a purely local free-dim reduction.
- All activations live TRANSPOSED on device: [features, cells]. Every GEMM is
  then out[m_feat_tile, cells] = W_slice.T @ actT with natural-layout weights
  as the PE stationary operand and N=256 moving (full-rate bf16).
- Host folds the two complex linears (o = a - g  =>  single weight diff Dr/Di)
  and turns the shared x-row into per-feature biases u_re/u_im.
- GRU inputs are reordered so cm = [x_mag(512), h_mag(1024), tension(1)]:
  12 clean K=128 tiles plus one K=1 rank-1 matmul for the tension feature.
- Device also computes softmax partials: e = exp(tension) (via the sigmoid
  identity, avoiding an extra ACT table load), per-core sum(e), sum(tension),
  and sum_c e_c * o[:, c].
- Host finishes the tiny cross-core pieces: debate blend (global mean),
  softmax normalization and the final [1,1024]x[1024,512] matvec.

All large DRAM tensors are pre-tiled on host to partition-major [128, T, N]
so every DMA moves >=4KB contiguous per partition (full HBM bandwidth).
"""

import numpy as np
import ml_dtypes

import concourse.bass as bass
import concourse.mybir as mybir
import concourse.tile as tile
from concourse.tile import ScopedClock
from concourse.bass_utils import run_bass_kernel_spmd

N_CELLS = 2048
D_IN = 512
D_HID = 1024
D_OUT = 512
SYNC = 0.15
DEBATE = 0.15
N_FACTIONS = 8
N_CORES = 8
C = N_CELLS // N_CORES  # 256 cells per core

F32 = mybir.dt.float32
AF = mybir.ActivationFunctionType
ALU = mybir.AluOpType

DT_MM = mybir.dt.bfloat16  # matmul dtype
NP_MM = ml_dtypes.bfloat16 if DT_MM == mybir.dt.bfloat16 else np.float32

KH = D_HID // 128   # 8 k-tiles over D_HID
MO = D_OUT // 128   # 4 m-tiles over D_OUT


class SplitDrainTileContext(tile.TileContext):
    """TileContext whose exit drain splits sem waits across multiple drain
    instructions (this walrus build allows only one sync wait per CTRL inst)."""

    def _drain_and_barrier(self, tick_clock, wait_clock):
        nc = self.nc
        drain_inst = nc.sync.drain()
        wait_clock.add_sem_waits(
            drain_inst.ins, ScopedClock({None: tick_clock.global_clock})
        )
        si = drain_inst.ins.sync_info
        waits = list(si.on_wait) if si is not None else []
        if len(waits) > 1:
            drain_inst.ins.sync_info = mybir.SyncInfo(
                on_wait=[waits[0]], on_update=[]
            )
            for w in waits[1:]:
                d2 = nc.sync.drain()
                d2.ins.sync_info = mybir.SyncInfo(on_wait=[w], on_update=[])
        nc.all_engine_barrier()
        assert self.sems is not None
        popped = nc._tile_sem_poison_stack.pop()
        assert popped is self._sem_poison
        nc.clear_and_free_semaphores(list(self.sems.allocated().values()))
        nc.all_engine_barrier()


def _split_multi_waits(nc):
    """This walrus build allows only one sync wait per instruction; hoist
    extra waits into wait-only EventSemaphore insts on the same engine stream
    (same program point, so semantics are identical for in-order engines)."""
    n = 0
    for fn in nc.m.functions:
        for bb in fn.blocks:
            lst = bb.instructions
            new = []
            for inst in lst:
                si = inst.sync_info
                if si is not None and len(si.on_wait) > 1:
                    waits = list(si.on_wait)
                    if type(inst).__name__ == "InstDMACopy":
                        # keep the queue's own ring-backpressure wait on the
                        # descriptor; hoist data-producer waits to issue time
                        keep_i = next(
                            (i for i in range(len(waits) - 1, -1, -1)
                             if (waits[i].ant_name or "").startswith("DMAHW")),
                            len(waits) - 1,
                        )
                    else:
                        keep_i = len(waits) - 1
                    keep = waits.pop(keep_i)
                    for w in waits:
                        n += 1
                        new.append(mybir.InstEventSemaphore(
                            name=f"I-splitw-{n}", engine=inst.engine,
                            ins=[], outs=[],
                            sync_info=mybir.SyncInfo(on_wait=[w], on_update=[])))
                    inst.sync_info = mybir.SyncInfo(
                        on_wait=[keep], on_update=list(si.on_update))
                new.append(inst)
            lst[:] = new
    return n


def build_nc():
    nc = bass.Bass(trn_type="TRN2")

    # ---- DRAM tensors (pre-tiled partition-major layouts) ----
    hreT = nc.dram_tensor("hreT", [128, KH, C], F32, kind="ExternalInput")
    himT = nc.dram_tensor("himT", [128, KH, C], F32, kind="ExternalInput")
    wdr = nc.dram_tensor("wdr", [128, KH, D_OUT], DT_MM, kind="ExternalInput")
    wdi = nc.dram_tensor("wdi", [128, KH, D_OUT], DT_MM, kind="ExternalInput")
    wzr = nc.dram_tensor("wzr", [128, 12, 2 * D_HID], DT_MM, kind="ExternalInput")
    wzr12d = nc.dram_tensor("wzr12", [1, 2 * D_HID], DT_MM, kind="ExternalInput")
    wgr = nc.dram_tensor("wgr", [128, 12, D_HID], DT_MM, kind="ExternalInput")
    wgr12d = nc.dram_tensor("wgr12", [1, D_HID], DT_MM, kind="ExternalInput")
    wgi = nc.dram_tensor("wgi", [128, 12, D_HID], DT_MM, kind="ExternalInput")
    biasp = nc.dram_tensor("biasp", [128, 40], F32, kind="ExternalInput")

    nhreT = nc.dram_tensor("nhreT", [128, KH, C], F32, kind="ExternalOutput")
    nhimT = nc.dram_tensor("nhimT", [128, KH, C], F32, kind="ExternalOutput")
    smalls = nc.dram_tensor("smalls", [128, 16], F32, kind="ExternalOutput")

    with SplitDrainTileContext(nc) as tc:
        with (
            tc.tile_pool(name="singles", bufs=1) as singles,
            tc.tile_pool(name="hmm", bufs=3) as hmm_pool,
            tc.tile_pool(name="wg", bufs=4) as wg_pool,
            tc.tile_pool(name="cand", bufs=3) as cand_pool,
            tc.tile_pool(name="work", bufs=3) as work,
            tc.tile_pool(name="rbuf", bufs=2) as rbuf,
            tc.tile_pool(name="bigscr", bufs=2) as bigscr,
            tc.tile_pool(name="psum", bufs=8, space="PSUM") as psum,
        ):
            # ---- GEMM1-critical loads first (f32 h; bf16 copies cast on-device) ----
            s_hre = singles.tile([128, KH, C], F32)
            nc.sync.dma_start(out=s_hre, in_=hreT[:, :, :])
            s_hrem = hmm_pool.tile([128, KH, C], DT_MM, tag="hmm")
            nc.vector.tensor_copy(s_hrem, s_hre[:, :, :])
            s_wdr = singles.tile([128, KH, D_OUT], DT_MM)
            nc.sync.dma_start(out=s_wdr[:, 0:4, :], in_=wdr[:, 0:4, :])
            nc.sync.dma_start(out=s_wdr[:, 4:KH, :], in_=wdr[:, 4:KH, :])
            s_him = singles.tile([128, KH, C], F32)
            nc.sync.dma_start(out=s_him, in_=himT[:, :, :])
            s_hnimm = hmm_pool.tile([128, KH, C], DT_MM, tag="hmm")
            nc.vector.tensor_scalar_mul(s_hnimm, s_him[:, :, :], -1.0)
            s_himm = hmm_pool.tile([128, KH, C], DT_MM, tag="hmm")
            nc.vector.tensor_copy(s_himm, s_him[:, :, :])
            s_wdi = singles.tile([128, KH, D_OUT], DT_MM)
            nc.sync.dma_start(out=s_wdi[:, 0:4, :], in_=wdi[:, 0:4, :])
            nc.sync.dma_start(out=s_wdi[:, 4:KH, :], in_=wdi[:, 4:KH, :])
            s_bias = singles.tile([128, 40], F32)
            nc.sync.dma_start(out=s_bias, in_=biasp[:, :])

            # z/r weights, chunked in GEMM2 consumption order (h_mag tiles first)
            s_wzr = singles.tile([128, 12, 2 * D_HID], DT_MM)
            nc.sync.dma_start(out=s_wzr[:, 4:8, :], in_=wzr[:, 4:8, :])
            nc.sync.dma_start(out=s_wzr[:, 8:12, :], in_=wzr[:, 8:12, :])
            nc.sync.dma_start(out=s_wzr[:, 0:4, :], in_=wzr[:, 0:4, :])
            s_wzr12 = singles.tile([1, 2 * D_HID], DT_MM)
            nc.sync.dma_start(out=s_wzr12, in_=wzr12d[:, :])

            ones_f32 = singles.tile([128, 1], F32)
            nc.vector.memset(ones_f32, 1.0)
            ones_row = singles.tile([1, 128], DT_MM)
            nc.vector.memset(ones_row, 1.0)

            # ---- h_mag squares on DVE (idle during GEMM1); sqrt on ACT later ----
            hmagm = singles.tile([128, KH, C], DT_MM)
            hm_halves = []
            for half in range(2):
                sl = slice(half * MO, (half + 1) * MO)
                hs1 = bigscr.tile([128, MO, C], F32, tag="bigscr")
                hs2 = bigscr.tile([128, MO, C], F32, tag="bigscr")
                nc.vector.tensor_mul(hs1, s_hre[:, sl, :], s_hre[:, sl, :])
                nc.vector.tensor_mul(hs2, s_him[:, sl, :], s_him[:, sl, :])
                nc.vector.tensor_add(hs1, hs1, hs2)
                hm_halves.append((sl, hs1))

            # ---- Phase A: engine GEMM -> oT [512, 256] re/im ----
            o_re = singles.tile([128, MO, C], F32)
            o_im = singles.tile([128, MO, C], F32)
            o_rem = singles.tile([128, MO, C], DT_MM)
            o_imm = singles.tile([128, MO, C], DT_MM)
            xmagm = singles.tile([128, MO, C], DT_MM)

            for m in range(MO):
                ms = slice(m * 128, (m + 1) * 128)
                p_re = psum.tile([128, C], F32, tag="ps")
                for t in range(KH):
                    nc.tensor.matmul(p_re, s_wdr[:, t, ms], s_hrem[:, t, :],
                                     start=(t == 0), stop=False)
                for t in range(KH):
                    nc.tensor.matmul(p_re, s_wdi[:, t, ms], s_hnimm[:, t, :],
                                     start=False, stop=(t == KH - 1))
                p_im = psum.tile([128, C], F32, tag="ps")
                for t in range(KH):
                    nc.tensor.matmul(p_im, s_wdr[:, t, ms], s_himm[:, t, :],
                                     start=(t == 0), stop=False)
                for t in range(KH):
                    nc.tensor.matmul(p_im, s_wdi[:, t, ms], s_hrem[:, t, :],
                                     start=False, stop=(t == KH - 1))
                # o = psum + u  (ACT Identity with per-partition bias)
                nc.scalar.activation(o_re[:, m, :], p_re, AF.Identity,
                                     bias=s_bias[:, m:m + 1])
                nc.scalar.activation(o_im[:, m, :], p_im, AF.Identity,
                                     bias=s_bias[:, 4 + m:5 + m])

            # h_mag sqrts (ACT, directly after the GEMM1 psum drains)
            for sl, hs1 in hm_halves:
                nc.scalar.activation(hmagm[:, sl, :], hs1, AF.Sqrt)

            # batched epilogue: casts, sq = o_re^2 + o_im^2, x_mag = sqrt(sq)
            nc.vector.tensor_copy(o_rem[:, :, :], o_re[:, :, :])
            nc.vector.tensor_copy(o_imm[:, :, :], o_im[:, :, :])
            osq1 = bigscr.tile([128, MO, C], F32, tag="bigscr")
            osq2 = bigscr.tile([128, MO, C], F32, tag="bigscr")
            nc.vector.tensor_mul(osq1, o_re[:, :, :], o_re[:, :, :])
            nc.scalar.activation(osq2, o_im[:, :, :], AF.Square)
            nc.vector.tensor_add(osq1, osq1, osq2)
            nc.scalar.activation(xmagm[:, :, :], osq1, AF.Sqrt)
            tension = singles.tile([1, C], F32)
            tension_m = singles.tile([1, C], DT_MM)

            # prefetch candidate weights (2 of 4 'wg' slots)
            wgr_a = wg_pool.tile([128, 6, D_HID], DT_MM, tag="wg")
            nc.sync.dma_start(out=wgr_a, in_=wgr[:, 0:6, :])
            wgr_b = wg_pool.tile([128, 6, D_HID], DT_MM, tag="wg")
            nc.sync.dma_start(out=wgr_b, in_=wgr[:, 6:12, :])
            s_wgr12 = singles.tile([1, D_HID], DT_MM)
            nc.sync.dma_start(out=s_wgr12, in_=wgr12d[:, :])

            # ---- Phase C: z | r GEMM over cm = [x_mag, h_mag, tension] ----
            z_f = singles.tile([128, KH, C], F32)
            rhrem = hmm_pool.tile([128, KH, C], DT_MM, tag="hmm")
            rhimm = hmm_pool.tile([128, KH, C], DT_MM, tag="hmm")
            # consume h_mag k-tiles first: x_mag (phase-A epilogue) lands later
            t_order = list(range(4, 12)) + list(range(0, 4))
            for m in range(16):
                ms = slice(m * 128, (m + 1) * 128)
                p = psum.tile([128, C], F32, tag="ps")
                for i, t in enumerate(t_order):
                    rhs = xmagm[:, t, :] if t < 4 else hmagm[:, t - 4, :]
                    nc.tensor.matmul(p, s_wzr[:, t, ms], rhs,
                                     start=(i == 0), stop=False,
                                     skip_group_check=True)
                if m == 0:
                    # tension = mean_feat(osq1): PE partition-sum (fp32),
                    # placed here so it never stalls the GEMM2 stream
                    p_t = psum.tile([1, C], F32, tag="ps")
                    for mm_i in range(MO):
                        nc.tensor.matmul(p_t, ones_f32, osq1[:, mm_i, :],
                                         start=(mm_i == 0),
                                         stop=(mm_i == MO - 1),
                                         skip_group_check=True)
                    nc.scalar.activation(tension, p_t, AF.Copy,
                                         scale=1.0 / D_OUT)
                    nc.vector.tensor_copy(tension_m, tension)
                nc.tensor.matmul(p, s_wzr12[:, ms], tension_m,
                                 start=False, stop=True,
                                 skip_group_check=True)
                if m < 8:
                    nc.scalar.activation(z_f[:, m, :], p, AF.Sigmoid,
                                         bias=s_bias[:, 8 + m:9 + m])
                else:
                    j = m - 8
                    r_t = rbuf.tile([128, C], F32, tag="rbuf")
                    nc.scalar.activation(r_t, p, AF.Sigmoid,
                                         bias=s_bias[:, 16 + j:17 + j])
                    # r*h in matmul dtype, immediately after r is ready
                    nc.vector.tensor_mul(rhrem[:, j, :], r_t, s_hre[:, j, :])
                    nc.vector.tensor_mul(rhimm[:, j, :], r_t, s_him[:, j, :])

            # ---- softmax partials (overlap with candidate GEMMs) ----
            # e = exp(tension) = s/(1-s), s = sigmoid(tension)
            sig = work.tile([1, C], F32, tag="tiny")
            om = work.tile([1, C], F32, tag="tiny")
            e_f = singles.tile([1, C], F32)
            nc.scalar.activation(sig, tension, AF.Sigmoid)
            nc.vector.tensor_scalar(om, sig, -1.0, 1.0, op0=ALU.mult, op1=ALU.add)
            nc.vector.reciprocal(om, om)
            nc.vector.tensor_mul(e_f, sig, om)
            e_m = singles.tile([1, C], DT_MM)
            nc.vector.tensor_copy(e_m, e_f)
            p_eb = psum.tile([128, C], F32, tag="ps")
            nc.tensor.matmul(p_eb, ones_row, e_m, start=True, stop=True)
            e_b = singles.tile([128, C], F32)
            nc.scalar.copy(e_b, p_eb)

            s_small = singles.tile([128, 16], F32)
            nc.vector.memset(s_small, 0.0)
            e_b_ap = e_b[:, :]
            e_b4 = bass.AP(tensor=e_b_ap.tensor, offset=e_b_ap.offset,
                           ap=[e_b_ap.ap[0], [0, MO], e_b_ap.ap[1]])
            wsum = bigscr.tile([128, MO, C], F32, tag="bigscr")
            nc.vector.tensor_mul(wsum, o_re[:, :, :], e_b4)
            nc.vector.reduce_sum(s_small[:, 0:MO], wsum,
                                 axis=mybir.AxisListType.X)
            wsum2 = bigscr.tile([128, MO, C], F32, tag="bigscr")
            nc.vector.tensor_mul(wsum2, o_im[:, :, :], e_b4)
            nc.vector.reduce_sum(s_small[:, MO:2 * MO], wsum2,
                                 axis=mybir.AxisListType.X)
            nc.vector.reduce_sum(s_small[0:1, 8:9], e_f,
                                 axis=mybir.AxisListType.X)
            nc.vector.reduce_sum(s_small[0:1, 9:10], tension,
                                 axis=mybir.AxisListType.X)

            # ---- Phase E+F: candidate GEMMs fused with the GRU/sync update ----
            fscale = singles.tile([128, 2 * KH], F32)

            def nh_update(comp, j, cand_t):
                h_t = s_hre if comp == 0 else s_him
                out_d = nhreT if comp == 0 else nhimT
                scr = work.tile([128, C], F32, tag="scr")
                nh = work.tile([128, C], F32, tag="nh")
                nc.vector.tensor_sub(scr, cand_t, h_t[:, j, :])
                nc.vector.tensor_mul(scr, z_f[:, j, :], scr)
                nc.vector.tensor_add(nh, h_t[:, j, :], scr)
                col = fscale[:, comp * KH + j:comp * KH + j + 1]
                nc.vector.reduce_sum(col, nh, axis=mybir.AxisListType.X)
                nc.vector.tensor_scalar_mul(col, col, SYNC / C)
                nc.vector.tensor_scalar(nh, nh, 1.0 - SYNC, col,
                                        op0=ALU.mult, op1=ALU.add)
                nc.sync.dma_start(out=out_d[:, j, :], in_=nh)

            wgi_tiles = []
            for m in range(KH):
                ms = slice(m * 128, (m + 1) * 128)
                p = psum.tile([128, C], F32, tag="ps")
                for t in range(12):
                    wt = wgr_a[:, t, ms] if t < 6 else wgr_b[:, t - 6, ms]
                    rhs = o_rem[:, t, :] if t < 4 else rhrem[:, t - 4, :]
                    nc.tensor.matmul(p, wt, rhs, start=(t == 0), stop=False)
                nc.tensor.matmul(p, s_wgr12[:, ms], tension_m,
                                 start=False, stop=True)
                cand_t = cand_pool.tile([128, C], F32, tag="cand")
                nc.scalar.activation(cand_t, p, AF.Tanh,
                                     bias=s_bias[:, 24 + m:25 + m])
                nh_update(0, m, cand_t)
                if m == 0:
                    # prefetch wgi into the two spare 'wg' slots
                    wgi_a = wg_pool.tile([128, 6, D_HID], DT_MM, tag="wg")
                    nc.sync.dma_start(out=wgi_a, in_=wgi[:, 0:6, :])
                    wgi_tiles.append(wgi_a)
                if m == 1:
                    wgi_b = wg_pool.tile([128, 6, D_HID], DT_MM, tag="wg")
                    nc.sync.dma_start(out=wgi_b, in_=wgi[:, 6:12, :])
                    wgi_tiles.append(wgi_b)

            for m in range(KH):
                ms = slice(m * 128, (m + 1) * 128)
                p = psum.tile([128, C], F32, tag="ps")
                for t in range(12):
                    wt = wgi_tiles[0][:, t, ms] if t < 6 else wgi_tiles[1][:, t - 6, ms]
                    rhs = o_imm[:, t, :] if t < 4 else rhimm[:, t - 4, :]
                    nc.tensor.matmul(p, wt, rhs, start=(t == 0), stop=(t == 11))
                cand_t = cand_pool.tile([128, C], F32, tag="cand")
                nc.scalar.activation(cand_t, p, AF.Tanh,
                                     bias=s_bias[:, 32 + m:33 + m])
                nh_update(1, m, cand_t)

            nc.sync.dma_start(out=smalls[:, :], in_=s_small)

    _split_multi_waits(nc)
    return nc


_NC_CACHE = {}


def _get_nc():
    if "nc" not in _NC_CACHE:
        _NC_CACHE["nc"] = build_nc()
    return _NC_CACHE["nc"]


def _tile8(a, np_dt):
    """[T*128, N] row-major -> partition-major [128, T, N] contiguous."""
    t = a.shape[0] // 128
    return np.ascontiguousarray(
        a.reshape(t, 128, a.shape[1]).transpose(1, 0, 2)
    ).astype(np_dt, copy=False)


def _prep_in_maps(inputs):
    f = {k: np.asarray(v, dtype=np.float32)
         for k, v in inputs.items() if k != "step"}
    x = f["x"]
    h_re = f["h_re"]
    h_im = f["h_im"]

    Dr = f["ea_wr"] - f["eg_wr"]
    Di = f["ea_wi"] - f["eg_wi"]
    dbr = f["ea_br"] - f["eg_br"]
    dbi = f["ea_bi"] - f["eg_bi"]
    u_re = (x @ Dr[:D_IN])[0] + dbr - dbi
    u_im = (x @ Di[:D_IN])[0] + dbr + dbi
    wdr = _tile8(Dr[D_IN:].astype(NP_MM), NP_MM)
    wdi = _tile8(Di[D_IN:].astype(NP_MM), NP_MM)

    def reorder(w):
        return np.concatenate([w[0:512], w[513:1537], w[512:513]], axis=0)

    wzr_full = reorder(np.concatenate([f["gz_w"], f["gr_w"]], axis=1)).astype(NP_MM)
    wgr_full = reorder(f["ghr_w"]).astype(NP_MM)
    wgi_full = np.concatenate(
        [f["ghi_w"][0:512], f["ghi_w"][513:1537]], axis=0).astype(NP_MM)

    biasp = np.zeros((128, 40), np.float32)
    for vec, c0 in [
        (u_re, 0), (u_im, 4),
        (f["gz_b"], 8), (f["gr_b"], 16), (f["ghr_b"], 24), (f["ghi_b"], 32),
    ]:
        nch = vec.shape[0] // 128
        biasp[:, c0:c0 + nch] = vec.reshape(nch, 128).T

    shared = dict(
        wdr=wdr, wdi=wdi,
        wzr=_tile8(wzr_full[0:1536], NP_MM),
        wzr12=np.ascontiguousarray(wzr_full[1536:1537]),
        wgr=_tile8(wgr_full[0:1536], NP_MM),
        wgr12=np.ascontiguousarray(wgr_full[1536:1537]),
        wgi=_tile8(wgi_full, NP_MM),
        biasp=biasp,
    )
    in_maps = []
    for c in range(N_CORES):
        hrT = np.ascontiguousarray(h_re[c * C:(c + 1) * C].T)  # [1024, 256]
        hiT = np.ascontiguousarray(h_im[c * C:(c + 1) * C].T)
        in_maps.append(dict(
            shared,
            hreT=_tile8(hrT, np.float32), himT=_tile8(hiT, np.float32),
        ))
    return in_maps


def _untile8(a):
    """[128, T, N] -> [T*128, N]."""
    return a.transpose(1, 0, 2).reshape(-1, a.shape[2])


def _assemble(inputs, results):
    step = int(np.asarray(inputs["step"]))
    oh_w = np.asarray(inputs["oh_w"], np.float32)
    oh_b = np.asarray(inputs["oh_b"], np.float32)

    nh_re = np.concatenate([_untile8(r["nhreT"]).T for r in results], axis=0)
    nh_im = np.concatenate([_untile8(r["nhimT"]).T for r in results], axis=0)
    if step > 5:
        dc = max(1, C // 4)
        for nh in (nh_re, nh_im):
            glob = nh.mean(axis=0, dtype=np.float32).astype(np.float32)
            for fct in range(N_FACTIONS):
                rows = slice(fct * C, fct * C + dc)
                nh[rows] = (1.0 - DEBATE) * nh[rows] + DEBATE * glob

    comb_re = np.zeros(512, np.float32)
    comb_im = np.zeros(512, np.float32)
    expsum = np.float32(0.0)
    tsum = np.float32(0.0)
    for r in results:
        s = r["smalls"]
        comb_re += s[:, 0:4].T.reshape(-1)
        comb_im += s[:, 4:8].T.reshape(-1)
        expsum += s[0, 8]
        tsum += s[0, 9]
    comb = np.concatenate([comb_re, comb_im]) / expsum
    pred = (comb[None, :] @ oh_w + oh_b).astype(np.float32)
    mt = np.float32(tsum / N_CELLS)
    return pred, mt, np.ascontiguousarray(nh_re, dtype=np.float32), \
        np.ascontiguousarray(nh_im, dtype=np.float32)


def _kernel_impl(inputs, trace=False, tmpdir=None):
    nc = _get_nc()
    in_maps = _prep_in_maps(inputs)
    res = run_bass_kernel_spmd(nc, in_maps, core_ids=list(range(N_CORES)),
                               trace=trace, tmpdir=tmpdir)
    return _assemble(inputs, res.results), res


def kernel(**inputs):
    outs, _ = _kernel_impl(inputs)
    return outs
